# revision 5
# baseline (speedup 1.0000x reference)
"""Masked multi-head attention on 8 Trainium2 NeuronCores.

Problem: B=2, H=12, S=2048, D=64 attention with an int32 {0,1} mask
broadcast over heads.  out = softmax(mask ? QK^T/8 : -inf) @ V.

Sharding (8 cores, no cross-core comm):
  core c -> (b = c>>2, head-group hg = (c>>1)&1 -> 6 heads, q-half qh = c&1
  -> 1024 queries).  Each core computes full attention (all 2048 keys) for
  its 6 heads x 1024 queries.

Host does all dtype/layout prep (fp16 conversion, pair-stacked K^T, V|ones,
mask^T as fp16 {0,1}) so the device runs zero conversion work, and the final
divide-by-denominator + [d,q]->[q,d] transpose also happen on host.

Per-core device algorithm (fp16 matmuls, fp32 accumulation):
  - The 6 heads x 16 k-tiles x 2 q-halves = 192 [128,512] score chunks form
    one flat stream, cut into 64 uniform [128,1536] PSUM tiles that ignore
    head boundaries.  ScalarE is the pacing engine (~1 elem/lane/cycle over
    all 12.6M score elements); 64 batched ACTIVATEs amortize the per-
    instruction overhead, and the uniform double-buffered tile stream has no
    per-head pool-reuse bubble.
  - scoresT[k, q] = K^T @ Q in [k (partitions), q (free)] layout.  The d=64
    contraction uses PE row-tiling: k-tile parity selects PE row group
    (0,0)/(64,0) so adjacent k-tiles stream concurrently.
  - exp on ScalarE straight from PSUM with the 1/8 scale fused.
  - mask: probs *= maskT slice (fp16 {0,1}) on VectorE (identical to -inf
    masking; a fully-masked row cannot occur with S=2048 random bits).
  - AV with V stationary: lhsT = [V_ktile | ones] (65 cols), rhs = streamed
    probsT [128k, 512q] -> out[d, q] accumulates over the 16 k-tiles in two
    single-bank PSUM accumulators; column 64 accumulates the softmax
    denominator for free.  This streams 512 useful columns per LDWEIGHTS
    instead of 65, cutting TensorE instruction count 4x vs probs-stationary.
  - AV for score-tile j is emitted after QK of tile j+2 so the in-order PE
    queue never blocks on a mask-DMA-gated tile while ScalarE starves.

PSUM budget (8 banks): scores 2x[128,1536] = 6, AV accumulators 2x[65,512]
= 2.
"""

import os
import sys

import numpy as np

for _p in ("/opt/trn_rl_repo",):
    if _p not in sys.path and os.path.isdir(_p):
        sys.path.insert(0, _p)

import concourse.bass as bass
import concourse.mybir as mybir
import concourse.tile as tile
from concourse import bacc
from concourse.bass_utils import run_bass_kernel_spmd

FP16 = mybir.dt.float16
F32 = mybir.dt.float32

B, H, S, D = 2, 12, 2048, 64
NCORES = 8
HPC = 6        # heads per core
QPC = 1024     # queries per core
KT = S // 128  # 16 k-tiles
CPH = 2 * KT   # 512-col score chunks per head (k-tile x q-half)
NCHUNK = HPC * CPH          # 192 chunks per core
TILECH = 3                  # chunks per score tile -> [128, 1536]
NTILE = NCHUNK // TILECH    # 64 score tiles
AVLAG = 2                   # score tiles between exp and its AV consumption

_NC_CACHE = None


def build_bass():
    """Build the single-core Bass/Tile program (SPMD across 8 cores)."""
    nc = bacc.Bacc("TRN2", target_bir_lowering=False, debug=False)

    qt = nc.declare_dram_parameter("qt", [HPC, 64, QPC], FP16, isOutput=False)
    kt = nc.declare_dram_parameter("kt", [HPC, 128, QPC], FP16, isOutput=False)
    vt = nc.declare_dram_parameter("vt", [HPC, 128, KT, 65], FP16, isOutput=False)
    mt = nc.declare_dram_parameter("mt", [KT, 128, QPC], FP16, isOutput=False)
    o = nc.declare_dram_parameter("o", [HPC, 65, QPC], F32, isOutput=True)

    with tile.TileContext(nc) as tc:
        with (
            tc.tile_pool(name="const", bufs=1) as const,
            tc.tile_pool(name="prp", bufs=12) as prp,
            tc.tile_pool(name="outp", bufs=2) as outp,
            tc.tile_pool(name="psc", bufs=2, space="PSUM") as psc,
            tc.tile_pool(name="pv0", bufs=1, space="PSUM") as pv0,
            tc.tile_pool(name="pv1", bufs=1, space="PSUM") as pv1,
        ):
            # Resident fp16 operands (loaded straight from DRAM, no casts).
            # qh: Q^T per head, duplicated on partitions 0-63 / 64-127 so both
            #     PE row groups can stream it.
            # kh: K^T per head pair-stacked: rows 0-63 hold even k-tiles,
            #     rows 64-127 odd k-tiles, 128 columns per k-tile pair.
            # vh: [V | ones] per (head, k-tile).
            # mk: mask^T as fp16 {0,1}, one tile per k-tile (fine-grained
            #     DMA-arrival deps).
            qh = const.tile([128, HPC, QPC], FP16)
            kh = const.tile([128, HPC, QPC], FP16)
            vh = const.tile([128, HPC, KT, 65], FP16)
            mk = [
                const.tile([128, QPC], FP16, name=f"mk{t}", tag=f"mk{t}")
                for t in range(KT)
            ]

            # DMA emission order controls arrival order.  Head 0's first
            # K-pair + Q gate the first QK; the mask tiles are consumed
            # throughout head 0, so they are interleaved just-in-time
            # between the per-head loads.
            nc.sync.dma_start(kh[:, 0, 0:128], kt[0][:, 0:128])
            nc.sync.dma_start(qh[0:64, 0, :], qt[0])
            nc.sync.dma_start(qh[64:128, 0, :], qt[0])
            nc.sync.dma_start(kh[:, 0, 128:QPC], kt[0][:, 128:QPC])
            nc.sync.dma_start(mk[0][:], mt[0])
            nc.sync.dma_start(mk[1][:], mt[1])
            nc.sync.dma_start(vh[:, 0, :, :], vt[0])
            nc.sync.dma_start(mk[2][:], mt[2])

            def load_head(h):
                nc.sync.dma_start(qh[0:64, h, :], qt[h])
                nc.sync.dma_start(qh[64:128, h, :], qt[h])
                nc.sync.dma_start(kh[:, h, :], kt[h])
                nc.sync.dma_start(vh[:, h, :, :], vt[h])

            load_head(1)
            for t in range(3, 6):
                nc.sync.dma_start(mk[t][:], mt[t])
            load_head(2)
            for t in range(6, 10):
                nc.sync.dma_start(mk[t][:], mt[t])
            load_head(3)
            for t in range(10, 14):
                nc.sync.dma_start(mk[t][:], mt[t])
            load_head(4)
            for t in range(14, 16):
                nc.sync.dma_start(mk[t][:], mt[t])
            load_head(5)

            avs = [None, None]  # per-q-half AV accumulators for current head
            av_h = [None]       # head owning avs

            def emit_epilogue():
                h = av_h[0]
                osb = outp.tile([65, QPC], F32, tag="os")
                nc.vector.tensor_copy(osb[:, 0:512], avs[0][:])
                nc.vector.tensor_copy(osb[:, 512:QPC], avs[1][:])
                nc.sync.dma_start(o[h], osb[:])
                avs[0] = avs[1] = None

            def emit_av(ent):
                """AV matmuls (and head epilogues) for a finished score tile."""
                pr, g0 = ent
                for ci in range(TILECH):
                    g = g0 + ci
                    h, cl = g // CPH, g % CPH
                    t, qc = cl // 2, cl % 2
                    if cl == 0 and avs[0] is not None:
                        emit_epilogue()
                    if t == 0:
                        pool = pv0 if qc == 0 else pv1
                        avs[qc] = pool.tile(
                            [65, 512], F32, name=f"av{qc}", tag="av"
                        )
                        av_h[0] = h
                    nc.tensor.matmul(
                        avs[qc][:],
                        vh[:, h, t, :],
                        pr[:, 512 * ci : 512 * (ci + 1)],
                        start=(t == 0),
                        stop=(t == KT - 1),
                    )

            pending = []
            for j in range(NTILE):
                g0 = TILECH * j
                sc = psc.tile([128, 512 * TILECH], F32, tag="sc")
                pr = prp.tile([128, 512 * TILECH], FP16, tag="pr")
                for ci in range(TILECH):
                    g = g0 + ci
                    h, cl = g // CPH, g % CPH
                    t, qc = cl // 2, cl % 2
                    r, a = t % 2, t // 2
                    nc.tensor.matmul(
                        sc[:, 512 * ci : 512 * (ci + 1)],
                        kh[64 * r : 64 * r + 64, h, 128 * a : 128 * a + 128],
                        qh[64 * r : 64 * r + 64, h, 512 * qc : 512 * (qc + 1)],
                        start=True,
                        stop=True,
                        tile_position=(64 * r, 0),
                    )
                # Tile j-AVLAG's AV lands in the PE queue here: its mask dep
                # is long satisfied, so the in-order PE stream never stalls.
                if len(pending) == AVLAG:
                    emit_av(pending.pop(0))
                nc.scalar.activation(
                    pr[:],
                    sc[:],
                    mybir.ActivationFunctionType.Exp,
                    scale=0.125,
                )
                # Mask multiplies: one tensor_mul per (head, k-tile) run
                # covered by this tile.
                ci = 0
                while ci < TILECH:
                    g = g0 + ci
                    h, cl = g // CPH, g % CPH
                    t, qc = cl // 2, cl % 2
                    n = 1
                    if qc == 0 and ci + 1 < TILECH:
                        n = 2  # second q-half of same k-tile follows
                    nc.vector.tensor_mul(
                        pr[:, 512 * ci : 512 * (ci + n)],
                        pr[:, 512 * ci : 512 * (ci + n)],
                        mk[t][:, 512 * qc : 512 * (qc + n)],
                    )
                    ci += n
                pending.append((pr, g0))
            for ent in pending:
                emit_av(ent)
            emit_epilogue()

    nc.compile()
    return nc


def _shard(c, Q, K, V, mask):
    b, hg, qhf = c >> 2, (c >> 1) & 1, c & 1
    hs = slice(hg * HPC, hg * HPC + HPC)
    qs = slice(qhf * QPC, qhf * QPC + QPC)
    # qt[h, d, q] = Q[b, h, qs+q, d]
    qtv = np.ascontiguousarray(Q[b, hs, qs, :].transpose(0, 2, 1)).astype(np.float16)
    # kt[h, 64r+d, 128a+cc] = K[b, h, 256a+128r+cc, d]  (pair-stacked K^T)
    kk = K[b, hs, :, :].reshape(HPC, KT // 2, 2, 128, 64).transpose(0, 2, 4, 1, 3)
    ktv = np.ascontiguousarray(kk).reshape(HPC, 128, QPC).astype(np.float16)
    # vt[h, p, t, 0:64] = V[b, h, 128t+p, :], col 64 = 1.0
    vtv = np.ones((HPC, 128, KT, 65), np.float16)
    vtv[..., 0:64] = V[b, hs, :, :].reshape(HPC, KT, 128, 64).transpose(0, 2, 1, 3)
    # mt[t, p, q] = mask[b, 0, qs+q, 128t+p]
    mtv = mask[b, 0, qs, :].T.reshape(KT, 128, QPC).astype(np.float16)
    return {"qt": qtv, "kt": ktv, "vt": vtv, "mt": mtv}


def get_nc():
    global _NC_CACHE
    if _NC_CACHE is None:
        _NC_CACHE = build_bass()
    return _NC_CACHE


def kernel(Q, K, V, mask):
    Q = np.asarray(Q, dtype=np.float32)
    K = np.asarray(K, dtype=np.float32)
    V = np.asarray(V, dtype=np.float32)
    mask = np.asarray(mask, dtype=np.int32)

    in_maps = [_shard(c, Q, K, V, mask) for c in range(NCORES)]
    res = run_bass_kernel_spmd(get_nc(), in_maps, list(range(NCORES))).results

    out = np.empty((B, H, S, D), dtype=np.float32)
    for c in range(NCORES):
        b, hg, qhf = c >> 2, (c >> 1) & 1, c & 1
        oc = res[c]["o"]  # [HPC, 65, QPC]: rows 0-63 = V-weighted sums, 64 = denom
        blk = (oc[:, 0:64, :] / oc[:, 64:65, :]).transpose(0, 2, 1)
        out[b, hg * HPC : hg * HPC + HPC, qhf * QPC : qhf * QPC + QPC, :] = blk
    return out
